# revision 10
# baseline (speedup 1.0000x reference)
"""Bass/Trainium2 kernel for nn_BayesianCTC (8-core data-parallel over batch).

Device (8 NeuronCores, 2 batch elements each): the O(B*T*V) bulk --
logits = hs_pad @ W.T + b, log-softmax LSE over V=2048, and the gathered
label/blank log-probs lp[b,t,0:201] (col 0 = blank, cols 1..200 = ys labels).
Host: the small O(B*T*S) CTC lattice forward/backward recursion in f64 numpy
(exact port of the reference), then the scalar loss.
"""

import numpy as np
import sys

sys.path.insert(0, "/opt/trn_rl_repo")

import concourse.bass as bass
import concourse.bacc as bacc_mod
import concourse.mybir as mybir
from concourse.tile import TileContext
from concourse import bass_utils

B, T, D, V, U = 16, 1600, 512, 2048, 200
NB = 2          # batch elems per core
NCORES = 8
L = U + 1       # blank + U labels
RISK_FACTOR = 0.1
NEG = float("-inf")
FP = mybir.dt.float32

_COMPILED = {}


def _build_bass():
    nc = bacc_mod.Bacc()

    KT = D // 128          # 4 k-tiles
    VC = V // 512          # 4 v-chunks
    # packed resident weights: [128, WCOLS] single DMA
    # cols: [0, KT*V): WT k-tiles | next NB*KT*L: WselT | 128: ones row |
    #       V: bias row | NB*L: bias-sel rows   (rows >0 zero where unused)
    OFS_WT = 0
    OFS_WS = KT * V
    OFS_ONES = OFS_WS + NB * KT * L
    OFS_B = OFS_ONES + 128
    OFS_BS = OFS_B + V
    WCOLS = OFS_BS + NB * L

    wpack = nc.dram_tensor("wpack", [128, WCOLS], FP, kind="ExternalInput")
    hsT = nc.dram_tensor("hsT", [NB * D, T], FP, kind="ExternalInput")
    lp_out = nc.dram_tensor("lp", [NB * T, L], FP, kind="ExternalOutput")

    n_full, rem = divmod(T, 128)
    tts = [128] * n_full + ([rem] if rem else [])

    with TileContext(nc) as tc:
        with (
            tc.tile_pool(name="wp", bufs=1) as wp_pool,
            tc.tile_pool(name="hs", bufs=3) as hs_pool,
            tc.tile_pool(name="scr", bufs=2) as scr_pool,
            tc.tile_pool(name="stat", bufs=3) as stat_pool,
            tc.tile_pool(name="lp", bufs=3) as lp_pool,
            tc.tile_pool(name="ps", bufs=2, space="PSUM") as ps_pool,
            tc.tile_pool(name="pslab", bufs=2, space="PSUM") as pslab_pool,
        ):
            wp = wp_pool.tile([128, WCOLS], FP, tag="wp")
            nc.sync.dma_start(wp[:], wpack[:, :])

            def wt_sl(k, vc):
                c = OFS_WT + k * V + vc * 512
                return wp[:, c:c + 512]

            def ws_sl(b, k):
                c = OFS_WS + (b * KT + k) * L
                return wp[:, c:c + L]

            for b in range(NB):
                for ti, tt in enumerate(tts):
                    t0 = ti * 128
                    hs4 = hs_pool.tile([128, KT * tt], FP, tag="hs4")
                    src = hsT[b * D: b * D + D, t0:t0 + tt].rearrange(
                        "(k p) t -> p k t", p=128)
                    dst = hs4[:].rearrange("p (k t) -> p k t", k=KT)
                    nc.sync.dma_start(dst, src)

                    ssums = stat_pool.tile([128, VC], FP, tag="ssums")
                    for vc in range(VC):
                        psum_v = ps_pool.tile([128, 512], FP, tag="psv")
                        for k in range(KT):
                            nc.tensor.matmul(
                                psum_v[:tt, :],
                                hs4[:, k * tt:(k + 1) * tt],
                                wt_sl(k, vc),
                                start=(k == 0), stop=False)
                        nc.tensor.matmul(
                            psum_v[:tt, :],
                            wp[0:1, OFS_ONES:OFS_ONES + tt],
                            wp[0:1, OFS_B + vc * 512:OFS_B + (vc + 1) * 512],
                            start=False, stop=True)
                        scr = scr_pool.tile([128, 512], FP, tag="scr")
                        nc.scalar.activation(
                            scr[:tt, :], psum_v[:tt, :],
                            mybir.ActivationFunctionType.Exp,
                            accum_out=ssums[:tt, vc:vc + 1])

                    # lse = log(sum of the 4 partial sums); neglse = -lse
                    ssum = stat_pool.tile([128, 1], FP, tag="ssum")
                    nc.vector.tensor_reduce(
                        ssum[:tt, :], ssums[:tt, :],
                        mybir.AxisListType.X, mybir.AluOpType.add)
                    neglse = stat_pool.tile([128, 1], FP, tag="neglse")
                    nc.scalar.activation(
                        neglse[:tt, :], ssum[:tt, :],
                        mybir.ActivationFunctionType.Ln)
                    nc.vector.tensor_scalar_mul(
                        neglse[:tt, :], neglse[:tt, :], -1.0)

                    # label logits -> lp = logits_sel - lse
                    psum_lab = pslab_pool.tile([128, L], FP, tag="pslab")
                    for k in range(KT):
                        nc.tensor.matmul(
                            psum_lab[:tt, :],
                            hs4[:, k * tt:(k + 1) * tt],
                            ws_sl(b, k),
                            start=(k == 0), stop=False)
                    nc.tensor.matmul(
                        psum_lab[:tt, :],
                        wp[0:1, OFS_ONES:OFS_ONES + tt],
                        wp[0:1, OFS_BS + b * L:OFS_BS + (b + 1) * L],
                        start=False, stop=True)
                    lp_tile = lp_pool.tile([128, L], FP, tag="lptile")
                    nc.scalar.activation(
                        lp_tile[:tt, :], psum_lab[:tt, :],
                        mybir.ActivationFunctionType.Identity,
                        bias=neglse[:tt, :])
                    nc.sync.dma_start(
                        lp_out[b * T + t0: b * T + t0 + tt, :], lp_tile[:tt, :])
    nc.compile()
    return nc


def _device_lp(hs_pad, W, bv, ysc):
    """Run the 8-core kernel; returns lp [B, T, L] f32."""
    key = "k"
    if key not in _COMPILED:
        _COMPILED[key] = _build_bass()
    nc = _COMPILED[key]

    KT = D // 128
    OFS_WS = KT * V
    OFS_ONES = OFS_WS + NB * KT * L
    OFS_B = OFS_ONES + 128
    OFS_BS = OFS_B + V
    WCOLS = OFS_BS + NB * L

    WT = np.ascontiguousarray(W.T, dtype=np.float32)          # [D, V]
    in_maps = []
    for c in range(NCORES):
        bs = [c * NB + i for i in range(NB)]
        wpack = np.zeros((128, WCOLS), dtype=np.float32)
        for k in range(KT):
            wpack[:, k * V:(k + 1) * V] = WT[k * 128:(k + 1) * 128, :]
        for i, b in enumerate(bs):
            Wsel = np.concatenate([W[0:1, :], W[ysc[b]]], axis=0)  # [L, D]
            WselT = Wsel.T                                          # [D, L]
            for k in range(KT):
                c0 = OFS_WS + (i * KT + k) * L
                wpack[:, c0:c0 + L] = WselT[k * 128:(k + 1) * 128, :]
            wpack[0, OFS_BS + i * L:OFS_BS + (i + 1) * L] = np.concatenate(
                [bv[0:1], bv[ysc[b]]])
        wpack[0, OFS_ONES:OFS_ONES + 128] = 1.0
        wpack[0, OFS_B:OFS_B + V] = bv
        hsT = np.ascontiguousarray(
            np.concatenate([hs_pad[b].T for b in bs], axis=0), dtype=np.float32)
        in_maps.append({"hsT": hsT, "wpack": wpack})

    res = bass_utils.run_bass_kernel_spmd(nc, in_maps, core_ids=list(range(NCORES)))
    lp = np.concatenate([r["lp"].reshape(NB, T, L) for r in res.results], axis=0)
    return lp


def _safe_lse0(x):
    m = np.max(x, axis=0)
    ms = np.where(np.isinf(m), 0.0, m)
    s = np.sum(np.exp(x - ms), axis=0)
    out = ms + np.log(np.where(s == 0, 1.0, s))
    return np.where(s == 0, NEG, out)


def _log_sub_exp(a, b):
    mask1 = (~np.isinf(a)) & (~np.isinf(b))
    a_ = np.where(mask1, a, -1.0)
    b_ = np.where(mask1, b, -2.0)
    tmp = b_ + np.log(np.exp(a_ - b_) - 1.0)
    a_ = np.where(np.isinf(tmp), -2000.0, a_)
    b_ = np.where(np.isinf(tmp), -2001.0, b_)
    ans1 = b_ + np.log(np.exp(a_ - b_) - 1.0)
    ans = np.where(mask1, ans1, NEG)
    ans = np.where((~np.isinf(a)) & np.isinf(b), a, ans)
    return ans


def _lattice_loss(lp, hlens, ys_pad):
    """f64 numpy port of the reference CTC-Bayes lattice given device lp."""
    Bn, Tn = B, T
    Un = U
    S = 2 * Un + 1
    lp = lp.astype(np.float64)
    ysc = np.where(ys_pad < 0, 0, ys_pad)
    olens = np.sum(ys_pad >= 0, axis=1)
    lp_blank = lp[:, :, 0]                       # [B,T]
    lp_label = lp[:, :, 1:]                      # [B,T,U]

    pair = np.stack([np.broadcast_to(lp_blank[:, :, None], (Bn, Tn, Un)),
                     lp_label], axis=-1).reshape(Bn, Tn, 2 * Un)
    em = np.concatenate([pair, lp_blank[:, :, None]], axis=-1)   # [B,T,S]
    allow_odd = np.concatenate(
        [np.zeros((Bn, 1), bool), ysc[:, 1:] != ysc[:, :-1]], axis=1)
    allow = np.concatenate(
        [np.stack([np.zeros((Bn, Un), bool), allow_odd], -1).reshape(Bn, 2 * Un),
         np.zeros((Bn, 1), bool)], axis=1)
    allow_fwd = np.concatenate([allow[:, 2:], np.zeros((Bn, 2), bool)], axis=1)

    em_t = np.transpose(em, (1, 0, 2))           # [T,B,S]
    ninf2 = np.full((Bn, S), NEG)
    alpha_all = np.empty((Tn, Bn, S))
    a = ninf2.copy()
    a[:, 0] = em_t[0, :, 0]
    a[:, 1] = em_t[0, :, 1]
    alpha_all[0] = a
    negcol1 = np.full((Bn, 1), NEG)
    negcol2 = np.full((Bn, 2), NEG)
    for t in range(1, Tn):
        s1 = np.concatenate([negcol1, a[:, :-1]], 1)
        s2 = np.concatenate([negcol2, a[:, :-2]], 1)
        s2 = np.where(allow, s2, NEG)
        a = em_t[t] + _safe_lse0(np.stack([a, s1, s2], 0))
        alpha_all[t] = a

    sidx = np.arange(S)[None, :]
    fin = np.where((sidx == 2 * olens[:, None]) |
                   (sidx == 2 * olens[:, None] - 1), 0.0, NEG)
    em_nxt = np.concatenate([em_t[1:], em_t[-1:]], 0)
    beta_all = np.empty((Tn, Bn, S))
    bcur = ninf2.copy()
    for t in range(Tn - 1, -1, -1):
        g = em_nxt[t] + bcur
        g1 = np.concatenate([g[:, 1:], negcol1], 1)
        g2 = np.concatenate([g[:, 2:], negcol2], 1)
        g2 = np.where(allow_fwd, g2, NEG)
        bcur = _safe_lse0(np.stack([g, g1, g2], 0))
        bcur = np.where((t == hlens - 1)[:, None], fin, bcur)
        beta_all[t] = bcur

    alpha_u = np.transpose(alpha_all, (1, 2, 0))[:, 1::2, :]     # [B,U,T]
    beta_u = np.transpose(beta_all, (1, 2, 0))[:, 1::2, :]
    valid = ((np.arange(Un)[None, :, None] < olens[:, None, None]) &
             (np.arange(Tn)[None, None, :] < hlens[:, None, None]))
    alpha_u = np.where(valid, alpha_u, NEG)
    beta_u = np.where(valid, beta_u, NEG)
    p = np.where(valid, np.transpose(lp_label, (0, 2, 1)), NEG)
    beta_prime = np.concatenate(
        [_log_sub_exp(beta_u[:, :, :-1], beta_u[:, :, 1:] + p[:, :, 1:]),
         beta_u[:, :, -1:]], axis=-1)
    risk = (np.arange(1, Tn + 1, dtype=np.float64)[None, None, :]
            / hlens[:, None, None].astype(np.float64) * RISK_FACTOR)
    loss_state = alpha_u + beta_prime + risk
    loss_state = np.where(np.isnan(loss_state), NEG, loss_state)
    m = np.max(loss_state, axis=2)
    ms = np.where(np.isinf(m), 0.0, m)
    ssum = np.sum(np.exp(loss_state - ms[:, :, None]), axis=2)
    loss_u = np.where(ssum == 0, NEG,
                      ms + np.log(np.where(ssum == 0, 1.0, ssum)))
    mask = np.isinf(loss_u)
    last = np.sum(~mask, axis=1) - 1
    loss_fsas = loss_u[np.arange(Bn), last]
    loss_fsas = np.where(hlens < olens, 0.0, loss_fsas)
    return np.mean(-loss_fsas)


def kernel(hs_pad, W, b, hlens, ys_pad, ali):
    hs_pad = np.asarray(hs_pad, dtype=np.float32)
    W = np.asarray(W, dtype=np.float32)
    bv = np.asarray(b, dtype=np.float32)
    hlens = np.asarray(hlens)
    ys_pad = np.asarray(ys_pad)
    ysc = np.where(ys_pad < 0, 0, ys_pad).astype(np.int64)

    with np.errstate(all="ignore"):
        lp = _device_lp(hs_pad, W, bv, ysc)
        loss = _lattice_loss(lp, hlens.astype(np.int64), ys_pad.astype(np.int64))
    return np.asarray(loss, dtype=np.float64)
